# revision 28
# baseline (speedup 1.0000x reference)
"""GQA causal-attention prefill kernel for Trainium2, tensor-parallel over 8 NeuronCores.

Reference semantics: q/k/v projections + RoPE + causal GQA attention +
output projection, B=2, T=2048, D=4096, 32 q heads, 8 kv heads, head_dim
128.  Core c owns q heads [4c, 4c+4), kv head c and the matching wo
slice; each core computes a full-shape partial output o_part and the
host sums the 8 partials (the tensor-parallel all-reduce).

Everything on the PE runs in bf16 (fp32 PSUM accumulation); measured
end-to-end max-rel error vs the fp32 reference is ~4e-3, well inside the
2e-2 gate, and bf16 halves DMA traffic, halves SBUF footprint (so all
weights + both batches' activations stay resident) and unlocks the
2-4x DVE 16-bit modes for the softmax bookkeeping.

Structure (emission order = engine program order):
  P1(b0), P1(b1):  projections + rope, TWO passes per batch over x
      (pass A: q0,q1,k; pass B: q2,q3,v).  3 accumulation groups x
      bufs=2 PSUM banks -> evictions of chunk c overlap the full 20us
      K-sweep of chunk c+1, so the PE never waits on a bank.  x is read
      twice (bf16 makes the 2x stream fit in HBM bandwidth); weights
      are loaded once up front, in k-group tiles so the first matmul
      only waits for ~1.5MB.
  P2(b0), P2(b1):  attention + o-projection per 512-token q-chunk.
      Scores transposed (sT = kT.T @ qT) so AV contracts s on the
      partition dim.  Softmax denominator comes from DVE adds of the
      exp tiles (off the PE) + ONE all-ones [128,128] matmul per
      (chunk, head) that sums over partitions AND broadcasts in one
      shot; 1/l via the fast custom-DVE reciprocal.  q-chunks are
      processed in pairs {3,0},{2,1} with heads interleaved so each
      stream's finalize chain hides behind a long stream's matmuls,
      and the o-projection of finished chunks is emitted between
      streams to keep the PE queue deep.
"""

import os
import sys

sys.path.insert(0, "/opt/trn_rl_repo")

import numpy as np

B = 2
T = 2048
TOK = B * T
D = 4096
NQ = 32
NKV = 8
H = 128
HH = H // 2
THETA = 10000.0
NCORES = 8
NHC = NQ // NCORES          # q heads per core (4)
KPC = D // H                # contraction chunks of 128 over D (32)
KG = 4                      # k-groups per weight tensor (8 chunks each)
TCH = 512                   # token chunk
NTCH = T // TCH             # 4 token chunks per batch
NSUB = TCH // H             # 4 s-subtiles per chunk
C_SM = 1.0 / np.sqrt(H)     # softmax scale


def _build_bass():
    import concourse.bacc as bacc
    import concourse.mybir as mybir
    import concourse.tile as tile
    from concourse.masks import make_identity
    from contextlib import ExitStack

    f32 = mybir.dt.float32
    bf16 = mybir.dt.bfloat16
    Exp = mybir.ActivationFunctionType.Exp
    Copy = mybir.ActivationFunctionType.Copy

    nc = bacc.Bacc("TRN2", target_bir_lowering=False, debug=False,
                   num_devices=NCORES)

    xT = nc.declare_dram_parameter("xT", [D, TOK], bf16, isOutput=False)
    # host pre-shuffled so every DMA row is >=2KB contiguous:
    # wqs[p, h, c, m] = wq[h, c*128+p, m]
    wqs = nc.declare_dram_parameter("wqs", [H, NHC, KPC, H], bf16,
                                    isOutput=False)
    wks = nc.declare_dram_parameter("wks", [H, KPC, H], bf16, isOutput=False)
    wvs = nc.declare_dram_parameter("wvs", [H, KPC, H], bf16, isOutput=False)
    # wos[p, h, d] = wo[h, p, d]
    wos = nc.declare_dram_parameter("wos", [H, NHC, D], bf16, isOutput=False)
    # rope tables duplicated across partition halves; sinT's TOP half is
    # NEGATED on the host so rope is out = direct*cosT + swap*sinT for all
    # 128 partitions in one mul+mul+add.
    cosT = nc.declare_dram_parameter("cosT", [H, TOK], bf16, isOutput=False)
    sinT = nc.declare_dram_parameter("sinT", [H, TOK], bf16, isOutput=False)
    o_part = nc.declare_dram_parameter("o_part", [TOK, D], bf16, isOutput=True)
    # x viewed as [p, kchunk, t] so one DMA start can fetch 4 k-chunks
    # (each dma_start costs ~600ns of serial Sync-sequencer time; the
    # un-batched version saturated that queue)
    xTv = xT.rearrange("(c p) t -> p c t", p=H)

    with tile.TileContext(nc) as tc:
        with ExitStack() as top:
            consts = top.enter_context(tc.tile_pool(name="consts", bufs=1))
            identity = consts.tile([H, H], bf16)
            make_identity(nc, identity)
            ones128 = consts.tile([H, H], bf16, tag="ones128")
            nc.vector.memset(ones128, 1.0)
            # 0/1 causal wedge for the 128x128 block that straddles the
            # diagonal: wedge[s, t'] = 1 iff t' >= s.  Blocks left of it are
            # skipped entirely (matmuls narrowed), blocks right of it are
            # all-ones (no mask needed).
            wedge = consts.tile([H, H], bf16, tag="wedge")
            nc.vector.memset(wedge, 1.0)
            nc.gpsimd.affine_select(
                out=wedge, in_=wedge,
                compare_op=mybir.AluOpType.is_ge,
                fill=0.0,
                base=0,
                pattern=[[1, H]],
                channel_multiplier=-1,
            )

            # ---- weights: loaded once, staged so x streaming stays ahead ----
            wpool = top.enter_context(tc.tile_pool(name="wpool", bufs=1))
            wq_t = [[wpool.tile([H, 8, H], bf16, tag=f"wq{h}_{g}",
                                name=f"wq{h}_{g}") for g in range(KG)]
                    for h in range(NHC)]
            wk_t = [wpool.tile([H, 8, H], bf16, tag=f"wk{g}", name=f"wk{g}")
                    for g in range(KG)]
            wv_t = [wpool.tile([H, 8, H], bf16, tag=f"wv{g}", name=f"wv{g}")
                    for g in range(KG)]
            wo_t = [wpool.tile([H, NHC, 1024], bf16, tag=f"wo{dq}",
                               name=f"wo{dq}") for dq in range(4)]
            cos_t = [wpool.tile([H, T], bf16, tag=f"cos{b}", name=f"cos{b}")
                     for b in range(B)]
            sin_t = [wpool.tile([H, T], bf16, tag=f"sin{b}", name=f"sin{b}")
                     for b in range(B)]

            # immediately needed: pass-A k-group 0; everything else is
            # drained between x loads so the first x tile isn't queued
            # behind megabytes of weights.  (The first x tile itself is
            # issued before even these, inside phase 1.)
            pend = []
            for g in range(1, KG):
                pend.append((wq_t[0][g], wqs[:, 0, g * 8:(g + 1) * 8, :]))
                pend.append((wq_t[1][g], wqs[:, 1, g * 8:(g + 1) * 8, :]))
                pend.append((wk_t[g], wks[:, g * 8:(g + 1) * 8, :]))
            pend.append((cos_t[0], cosT[:, 0:T]))
            pend.append((sin_t[0], sinT[:, 0:T]))
            for g in range(KG):
                pend.append((wq_t[2][g], wqs[:, 2, g * 8:(g + 1) * 8, :]))
                pend.append((wq_t[3][g], wqs[:, 3, g * 8:(g + 1) * 8, :]))
                pend.append((wv_t[g], wvs[:, g * 8:(g + 1) * 8, :]))
            pend.append((cos_t[1], cosT[:, T:TOK]))
            pend.append((sin_t[1], sinT[:, T:TOK]))
            for dq in range(4):
                pend.append((wo_t[dq], wos[:, :, dq * 1024:(dq + 1) * 1024]))

            def drain_pend(n):
                for _ in range(n):
                    if pend:
                        dst, src = pend.pop(0)
                        nc.sync.dma_start(out=dst, in_=src)

            # ---- activations, both batches resident (bf16) ----
            apool = top.enter_context(tc.tile_pool(name="apool", bufs=1))
            qTs = [[apool.tile([H, NHC, TCH], bf16, tag=f"qT{b}_{i}",
                               name=f"qT{b}_{i}") for i in range(NTCH)]
                   for b in range(B)]
            kTs = [[apool.tile([H, TCH], bf16, tag=f"kT{b}_{i}",
                               name=f"kT{b}_{i}") for i in range(NTCH)]
                   for b in range(B)]
            vs = [[apool.tile([H, NSUB, H], bf16, tag=f"v{b}_{i}",
                              name=f"v{b}_{i}") for i in range(NTCH)]
                  for b in range(B)]

            # ================= phase 1: projections + rope =================
            with ExitStack() as ph1:
                xpool = ph1.enter_context(tc.tile_pool(name="xpool", bufs=5))
                rtmp = ph1.enter_context(tc.tile_pool(name="rtmp", bufs=2))
                pj = ph1.enter_context(
                    tc.tile_pool(name="pj", bufs=2, space="PSUM"))
                pt = ph1.enter_context(
                    tc.tile_pool(name="pt", bufs=2, space="PSUM"))

                # very first x tile goes out ahead of all weight DMAs
                x0_t = xpool.tile([H, 8, TCH], bf16, tag="x", name="x_t")
                nc.sync.dma_start(out=x0_t, in_=xTv[:, 0:8, 0:TCH])
                nc.sync.dma_start(out=wq_t[0][0], in_=wqs[:, 0, 0:8, :])
                nc.sync.dma_start(out=wq_t[1][0], in_=wqs[:, 1, 0:8, :])
                nc.sync.dma_start(out=wk_t[0], in_=wks[:, 0:8, :])

                def rope_from_psum(psum, dst_ap, cs, sn):
                    # swap staging: halves exchanged so the mul against the
                    # (half-duplicated) rope table is one full-width op.
                    swap = rtmp.tile([H, TCH], f32, tag="swap", bufs=3,
                                     name="swap")
                    nc.vector.tensor_copy(swap[0:HH, :], psum[HH:H, :])
                    nc.vector.tensor_copy(swap[HH:H, :], psum[0:HH, :])
                    m1 = rtmp.tile([H, TCH], f32, tag="m1", name="m1")
                    m2 = rtmp.tile([H, TCH], f32, tag="m2", name="m2")
                    nc.vector.tensor_mul(m1, psum, cs)
                    nc.vector.tensor_mul(m2, swap, sn)
                    nc.vector.tensor_add(dst_ap, m1, m2)

                # staged weight-DMA drain counts, interleaved between the
                # x loads of batch 0 (emission precedes every consumer —
                # Tile deps follow emission order; kg g's weights are
                # drained right before the x group that consumes them).
                drains = {0: [[0, 3, 3, 5], [0, 4, 0, 0], [0, 4, 0, 0],
                              [0, 4, 0, 0]],
                          1: [[0, 2, 0, 0], [0, 2, 0, 0], [0, 2, 0, 0],
                              [0, 0, 0, 0]]}
                for b in range(B):
                    tb = b * T
                    for pas in range(2):
                        for tch in range(NTCH):
                            t0 = tch * TCH
                            g_ps = [pj.tile([H, TCH], f32, tag=f"g{i}",
                                            name=f"g{i}") for i in range(3)]
                            for kq in range(KPC // 8):
                                if b == 0:
                                    drain_pend(drains[pas][tch][kq])
                                if b == 0 and pas == 0 and tch == 0 \
                                        and kq == 0:
                                    x_t = x0_t
                                else:
                                    x_t = xpool.tile([H, 8, TCH], bf16,
                                                     tag="x", name="x_t")
                                    nc.sync.dma_start(
                                        out=x_t,
                                        in_=xTv[:, kq * 8:(kq + 1) * 8,
                                                tb + t0:tb + t0 + TCH])
                                for kc in range(8):
                                    k = kq * 8 + kc
                                    if pas == 0:
                                        lhs = [wq_t[0][kq][:, kc, :],
                                               wq_t[1][kq][:, kc, :],
                                               wk_t[kq][:, kc, :]]
                                    else:
                                        lhs = [wq_t[2][kq][:, kc, :],
                                               wq_t[3][kq][:, kc, :],
                                               wv_t[kq][:, kc, :]]
                                    for gi in range(3):
                                        nc.tensor.matmul(
                                            g_ps[gi], lhs[gi], x_t[:, kc, :],
                                            start=(k == 0),
                                            stop=(k == KPC - 1),
                                            skip_group_check=True)
                            cs = cos_t[b][:, t0:t0 + TCH]
                            sn = sin_t[b][:, t0:t0 + TCH]
                            if pas == 0:
                                rope_from_psum(g_ps[2], kTs[b][tch], cs, sn)
                                rope_from_psum(g_ps[0], qTs[b][tch][:, 0, :],
                                               cs, sn)
                                rope_from_psum(g_ps[1], qTs[b][tch][:, 1, :],
                                               cs, sn)
                            else:
                                # q-ropes first: their PSUM banks free
                                # sooner, which is what phase 2's first
                                # scores wait on at the P1->P2 boundary
                                rope_from_psum(g_ps[0], qTs[b][tch][:, 2, :],
                                               cs, sn)
                                rope_from_psum(g_ps[1], qTs[b][tch][:, 3, :],
                                               cs, sn)
                                vstage = rtmp.tile([H, TCH], bf16,
                                                   tag="vstage", name="vstage")
                                nc.vector.tensor_copy(vstage, g_ps[2])
                                for j in range(NSUB):
                                    tp = pt.tile([H, H], bf16, tag="vtp",
                                                 name="vtp")
                                    nc.tensor.transpose(
                                        tp, vstage[:, j * H:(j + 1) * H],
                                        identity)
                                    nc.vector.tensor_copy(
                                        vs[b][tch][:, j, :], tp)

            # ============= phase 2: attention + o-projection =============
            with ExitStack() as ph2:
                p2pool = ph2.enter_context(tc.tile_pool(name="p2pool", bufs=4))
                lpool = ph2.enter_context(tc.tile_pool(name="lpool", bufs=2))
                rpool = ph2.enter_context(tc.tile_pool(name="rpool", bufs=2))
                otpool = ph2.enter_context(tc.tile_pool(name="otpool", bufs=1))
                opool = ph2.enter_context(tc.tile_pool(name="opool", bufs=2))
                ps_s = ph2.enter_context(
                    tc.tile_pool(name="ps_s", bufs=3, space="PSUM"))
                ps_av = ph2.enter_context(
                    tc.tile_pool(name="ps_av", bufs=2, space="PSUM"))
                ps_lbc = ph2.enter_context(
                    tc.tile_pool(name="ps_lbc", bufs=1, space="PSUM"))
                ps_o = ph2.enter_context(
                    tc.tile_pool(name="ps_o", bufs=2, space="PSUM"))

                def attn_stream(b, outTs, qc, h):
                    """Emit one (q-chunk, head) stream: scores/AV matmuls
                    plus finalize (denominator broadcast + reciprocal +
                    normalize).  The final lsum add is a narrow diagonal
                    tile, so the broadcast matmul never waits on DVE."""
                    n_st = (qc + 1) * NSUB
                    rhs_q = qTs[b][qc][:, h, :]
                    av_ps = ps_av.tile([H, TCH], f32, tag="av",
                                       name="av_ps")
                    lsum = lpool.tile([H, TCH], bf16, tag="lsum",
                                      name="lsum")

                    def scores_block(st):
                        # diagonal-band tiles are narrowed to the causally
                        # reachable columns t >= j*128; only the 128-wide
                        # block straddling the diagonal needs masking
                        j = st - qc * NSUB
                        nw = j * H if j > 0 else 0
                        sps = ps_s.tile([H, TCH], f32, tag="s", name="sps")
                        kt = kTs[b][st // NSUB][
                            :, (st % NSUB) * H:(st % NSUB + 1) * H]
                        nc.tensor.matmul(sps[:, nw:], kt, rhs_q[:, nw:],
                                         start=True, stop=True)
                        pT2 = p2pool.tile([H, TCH], bf16, tag="p2",
                                          name="pT2")
                        nc.scalar.activation(pT2[:, nw:], sps[:, nw:], Exp,
                                             scale=C_SM)
                        if j >= 0:
                            nc.vector.tensor_mul(pT2[:, nw:nw + H],
                                                 pT2[:, nw:nw + H], wedge)
                        # softmax denominator accumulates on DVE, off the
                        # PE's critical path
                        if st == 0:
                            nc.vector.tensor_copy(lsum, pT2)
                        else:
                            nc.vector.tensor_add(lsum[:, nw:], lsum[:, nw:],
                                                 pT2[:, nw:])
                        return pT2, nw

                    def av_block(st, pT2, nw):
                        nc.tensor.matmul(
                            av_ps[:, nw:],
                            vs[b][st // NSUB][:, st % NSUB, :], pT2[:, nw:],
                            start=(st == 0), stop=(st == n_st - 1),
                            skip_group_check=True)

                    # lookahead-2: two score blocks in flight ahead of each
                    # AV so the exp/mask latency never stalls the PE
                    pending = [scores_block(0), scores_block(1)]
                    for st in range(2, n_st):
                        pending.append(scores_block(st))
                        av_block(st - 2, *pending.pop(0))
                    av_block(n_st - 2, *pending.pop(0))
                    av_block(n_st - 1, *pending.pop(0))
                    # partition-sum + broadcast of the denominator in one
                    # all-ones matmul, then fast reciprocal + normalize
                    lbc = ps_lbc.tile([H, TCH], f32, tag="lbc", name="lbc")
                    nc.tensor.matmul(lbc, ones128, lsum,
                                     start=True, stop=True)
                    rl = rpool.tile([H, TCH], f32, tag="rl", name="rl")
                    nc.vector.reciprocal_approx_fast(out=rl, in_=lbc)
                    nc.vector.tensor_mul(outTs[qc][:, h, :], av_ps, rl)

                def oproj_units(b, outTs, qc):
                    """o-projection of one q-chunk as 8 independent thunks
                    (one per (u, dh)), drained between attention streams to
                    keep the PE queue deep."""
                    tb = b * T
                    outT = outTs[qc]
                    units = []
                    for u in range(NSUB):
                        for dh in range(2):
                            def unit(u=u, dh=dh):
                                trow = tb + qc * TCH + u * H
                                # 4 PSUM evictions batched into one 4KB-row
                                # store: keeps the Sync queue off the
                                # critical path
                                o_sb = opool.tile([H, 4, TCH], bf16,
                                                  tag="osb", name="o_sb")
                                for j in range(4):
                                    dc = dh * 4 + j
                                    ops = ps_o.tile([H, TCH], f32, tag="o",
                                                    name="ops")
                                    for h in range(NHC):
                                        nc.tensor.matmul(
                                            ops,
                                            outT[:, h, u * H:(u + 1) * H],
                                            wo_t[dc // 2][:, h,
                                                          (dc % 2) * TCH:
                                                          (dc % 2 + 1) * TCH],
                                            start=(h == 0),
                                            stop=(h == NHC - 1),
                                            skip_group_check=True)
                                    nc.scalar.activation(
                                        o_sb[:, j, :], ops, Copy)
                                nc.sync.dma_start(
                                    out=o_part[trow:trow + H,
                                               dh * 2048:(dh + 1) * 2048],
                                    in_=o_sb)
                            units.append(unit)
                    return units

                opq = []        # pending o-proj units, carried across batches
                for b in range(B):
                    outTs = {qc: otpool.tile([H, NHC, TCH], bf16,
                                             tag=f"outT{qc}",
                                             name=f"outT{qc}")
                             for qc in range(NTCH)}
                    # qc3's four streams first (long streams hide each
                    # other's finalize), then qc0's short streams padded by
                    # qc3's o-projection, then the {2,1} pair alternated so
                    # qc1's short streams hide behind qc2's long ones.
                    streams = [(3, h) for h in range(NHC)]
                    streams += [(0, h) for h in range(NHC)]
                    for h in range(NHC):
                        streams.append((2, h))
                        streams.append((1, h))
                    for i, (qc, h) in enumerate(streams):
                        attn_stream(b, outTs, qc, h)
                        if i == 3:
                            opq += oproj_units(b, outTs, 3)
                        elif i == 7:
                            opq += oproj_units(b, outTs, 0)
                        for _ in range(2):
                            if opq:
                                opq.pop(0)()
                    opq += oproj_units(b, outTs, 2)
                    opq += oproj_units(b, outTs, 1)
                for unit in opq:
                    unit()

    nc.compile()
    return nc


_NC_CACHE = None


def _prep_inputs(x, wq, wk, wv, wo, positions):
    import ml_dtypes
    bf = ml_dtypes.bfloat16

    x = np.asarray(x, dtype=np.float32)
    wq = np.asarray(wq, dtype=np.float32)
    wk = np.asarray(wk, dtype=np.float32)
    wv = np.asarray(wv, dtype=np.float32)
    wo = np.asarray(wo, dtype=np.float32)
    positions = np.asarray(positions)

    xT = np.ascontiguousarray(x.reshape(TOK, D).T.astype(bf))
    # rope tables [H, TOK], duplicated across halves, sin top half negated
    fraction = 2.0 * np.arange(HH, dtype=np.float32) / H
    timescale = (THETA ** fraction).astype(np.float32)
    pos = positions.reshape(TOK).astype(np.float32)
    sinu = pos[None, :] / timescale[:, None]
    cos = np.cos(sinu).astype(np.float32)
    sin = np.sin(sinu).astype(np.float32)
    cosT = np.ascontiguousarray(np.concatenate([cos, cos], 0).astype(bf))
    sinT = np.ascontiguousarray(np.concatenate([-sin, sin], 0).astype(bf))

    in_maps = []
    for c in range(NCORES):
        wq_c = wq[c * NHC:(c + 1) * NHC]            # [4, D, H]
        wqs = np.ascontiguousarray(
            wq_c.reshape(NHC, KPC, H, H).transpose(2, 0, 1, 3).astype(bf))
        wks = np.ascontiguousarray(
            wk[c].reshape(KPC, H, H).transpose(1, 0, 2).astype(bf))
        wvs = np.ascontiguousarray(
            wv[c].reshape(KPC, H, H).transpose(1, 0, 2).astype(bf))
        wos = np.ascontiguousarray(
            wo[c * NHC:(c + 1) * NHC].transpose(1, 0, 2).astype(bf))
        in_maps.append({
            "xT": xT,
            "wqs": wqs,
            "wks": wks,
            "wvs": wvs,
            "wos": wos,
            "cosT": cosT,
            "sinT": sinT,
        })
    return in_maps


def kernel(x, wq, wk, wv, wo, positions):
    global _NC_CACHE
    from concourse.bass_utils import run_bass_kernel_spmd

    in_maps = _prep_inputs(x, wq, wk, wv, wo, positions)

    if _NC_CACHE is None:
        _NC_CACHE = _build_bass()
    nc = _NC_CACHE

    trace = os.environ.get("BASS_KERNEL_TRACE", "0") == "1"
    res = run_bass_kernel_spmd(nc, in_maps, list(range(NCORES)), trace=trace)
    global LAST_RESULTS
    LAST_RESULTS = res
    out = np.zeros((TOK, D), dtype=np.float32)
    for c in range(NCORES):
        out += np.asarray(res.results[c]["o_part"]).astype(np.float32)
    return out.reshape(B, T, D)


LAST_RESULTS = None


# revision 30
# speedup vs baseline: 1.0036x; 1.0036x over previous
"""GQA causal-attention prefill kernel for Trainium2, tensor-parallel over 8 NeuronCores.

Reference semantics: q/k/v projections + RoPE + causal GQA attention +
output projection, B=2, T=2048, D=4096, 32 q heads, 8 kv heads, head_dim
128.  Core c owns q heads [4c, 4c+4), kv head c and the matching wo
slice; each core computes a full-shape partial output o_part and the
host sums the 8 partials (the tensor-parallel all-reduce).

Everything on the PE runs in bf16 (fp32 PSUM accumulation); measured
end-to-end max-rel error vs the fp32 reference is ~4e-3, well inside the
2e-2 gate, and bf16 halves DMA traffic, halves SBUF footprint (so all
weights + both batches' activations stay resident) and unlocks the
2-4x DVE 16-bit modes for the softmax bookkeeping.

Structure (emission order = engine program order):
  P1(b0), P1(b1):  projections + rope, TWO passes per batch over x
      (pass A: q0,q1,k; pass B: q2,q3,v).  3 accumulation groups x
      bufs=2 PSUM banks -> evictions of chunk c overlap the full 20us
      K-sweep of chunk c+1, so the PE never waits on a bank.  x is read
      twice (bf16 makes the 2x stream fit in HBM bandwidth); weights
      are loaded once up front, in k-group tiles so the first matmul
      only waits for ~1.5MB.
  P2(b0), P2(b1):  attention + o-projection per 512-token q-chunk.
      Scores transposed (sT = kT.T @ qT) so AV contracts s on the
      partition dim.  Softmax denominator comes from DVE adds of the
      exp tiles (off the PE) + ONE all-ones [128,128] matmul per
      (chunk, head) that sums over partitions AND broadcasts in one
      shot; 1/l via the fast custom-DVE reciprocal.  q-chunks are
      processed in pairs {3,0},{2,1} with heads interleaved so each
      stream's finalize chain hides behind a long stream's matmuls,
      and the o-projection of finished chunks is emitted between
      streams to keep the PE queue deep.
"""

import os
import sys

sys.path.insert(0, "/opt/trn_rl_repo")

import numpy as np

B = 2
T = 2048
TOK = B * T
D = 4096
NQ = 32
NKV = 8
H = 128
HH = H // 2
THETA = 10000.0
NCORES = 8
NHC = NQ // NCORES          # q heads per core (4)
KPC = D // H                # contraction chunks of 128 over D (32)
KG = 4                      # k-groups per weight tensor (8 chunks each)
TCH = 512                   # token chunk
NTCH = T // TCH             # 4 token chunks per batch
NSUB = TCH // H             # 4 s-subtiles per chunk
C_SM = 1.0 / np.sqrt(H)     # softmax scale


def _build_bass():
    import concourse.bacc as bacc
    import concourse.mybir as mybir
    import concourse.tile as tile
    from concourse.masks import make_identity
    from contextlib import ExitStack

    f32 = mybir.dt.float32
    bf16 = mybir.dt.bfloat16
    Exp = mybir.ActivationFunctionType.Exp
    Copy = mybir.ActivationFunctionType.Copy

    nc = bacc.Bacc("TRN2", target_bir_lowering=False, debug=False,
                   num_devices=NCORES)

    xT = nc.declare_dram_parameter("xT", [D, TOK], bf16, isOutput=False)
    # host pre-shuffled so every DMA row is >=2KB contiguous:
    # wqs[p, h, c, m] = wq[h, c*128+p, m]
    wqs = nc.declare_dram_parameter("wqs", [H, NHC, KPC, H], bf16,
                                    isOutput=False)
    wks = nc.declare_dram_parameter("wks", [H, KPC, H], bf16, isOutput=False)
    wvs = nc.declare_dram_parameter("wvs", [H, KPC, H], bf16, isOutput=False)
    # wos[p, h, d] = wo[h, p, d]
    wos = nc.declare_dram_parameter("wos", [H, NHC, D], bf16, isOutput=False)
    # rope tables duplicated across partition halves; sinT's TOP half is
    # NEGATED on the host so rope is out = direct*cosT + swap*sinT for all
    # 128 partitions in one mul+mul+add.
    cosT = nc.declare_dram_parameter("cosT", [H, TOK], bf16, isOutput=False)
    sinT = nc.declare_dram_parameter("sinT", [H, TOK], bf16, isOutput=False)
    o_part = nc.declare_dram_parameter("o_part", [TOK, D], bf16, isOutput=True)
    # x viewed as [p, kchunk, t] so one DMA start can fetch 4 k-chunks
    # (each dma_start costs ~600ns of serial Sync-sequencer time; the
    # un-batched version saturated that queue)
    xTv = xT.rearrange("(c p) t -> p c t", p=H)

    with tile.TileContext(nc) as tc:
        with ExitStack() as top:
            consts = top.enter_context(tc.tile_pool(name="consts", bufs=1))
            identity = consts.tile([H, H], bf16)
            make_identity(nc, identity)
            ones128 = consts.tile([H, H], bf16, tag="ones128")
            nc.vector.memset(ones128, 1.0)
            # 0/1 causal wedge for the 128x128 block that straddles the
            # diagonal: wedge[s, t'] = 1 iff t' >= s.  Blocks left of it are
            # skipped entirely (matmuls narrowed), blocks right of it are
            # all-ones (no mask needed).
            wedge = consts.tile([H, H], bf16, tag="wedge")
            nc.vector.memset(wedge, 1.0)
            nc.gpsimd.affine_select(
                out=wedge, in_=wedge,
                compare_op=mybir.AluOpType.is_ge,
                fill=0.0,
                base=0,
                pattern=[[1, H]],
                channel_multiplier=-1,
            )

            # ---- weights: loaded once, staged so x streaming stays ahead ----
            wpool = top.enter_context(tc.tile_pool(name="wpool", bufs=1))
            wq_t = [[wpool.tile([H, 8, H], bf16, tag=f"wq{h}_{g}",
                                name=f"wq{h}_{g}") for g in range(KG)]
                    for h in range(NHC)]
            wk_t = [wpool.tile([H, 8, H], bf16, tag=f"wk{g}", name=f"wk{g}")
                    for g in range(KG)]
            wv_t = [wpool.tile([H, 8, H], bf16, tag=f"wv{g}", name=f"wv{g}")
                    for g in range(KG)]
            wo_t = [wpool.tile([H, NHC, 1024], bf16, tag=f"wo{dq}",
                               name=f"wo{dq}") for dq in range(4)]
            cos_t = [wpool.tile([H, T], bf16, tag=f"cos{b}", name=f"cos{b}")
                     for b in range(B)]
            sin_t = [wpool.tile([H, T], bf16, tag=f"sin{b}", name=f"sin{b}")
                     for b in range(B)]

            # immediately needed: pass-A k-group 0; everything else is
            # drained between x loads so the first x tile isn't queued
            # behind megabytes of weights.  (The first x tile itself is
            # issued before even these, inside phase 1.)
            pend = []
            for g in range(1, KG):
                pend.append((wq_t[0][g], wqs[:, 0, g * 8:(g + 1) * 8, :]))
                pend.append((wq_t[1][g], wqs[:, 1, g * 8:(g + 1) * 8, :]))
                pend.append((wk_t[g], wks[:, g * 8:(g + 1) * 8, :]))
            pend.append((cos_t[0], cosT[:, 0:T]))
            pend.append((sin_t[0], sinT[:, 0:T]))
            for g in range(KG):
                pend.append((wq_t[2][g], wqs[:, 2, g * 8:(g + 1) * 8, :]))
                pend.append((wq_t[3][g], wqs[:, 3, g * 8:(g + 1) * 8, :]))
                pend.append((wv_t[g], wvs[:, g * 8:(g + 1) * 8, :]))
            pend.append((cos_t[1], cosT[:, T:TOK]))
            pend.append((sin_t[1], sinT[:, T:TOK]))
            for dq in range(4):
                pend.append((wo_t[dq], wos[:, :, dq * 1024:(dq + 1) * 1024]))

            def drain_pend(n):
                for _ in range(n):
                    if pend:
                        dst, src = pend.pop(0)
                        nc.sync.dma_start(out=dst, in_=src)

            # ---- activations, both batches resident (bf16) ----
            apool = top.enter_context(tc.tile_pool(name="apool", bufs=1))
            qTs = [[apool.tile([H, NHC, TCH], bf16, tag=f"qT{b}_{i}",
                               name=f"qT{b}_{i}") for i in range(NTCH)]
                   for b in range(B)]
            kTs = [[apool.tile([H, TCH], bf16, tag=f"kT{b}_{i}",
                               name=f"kT{b}_{i}") for i in range(NTCH)]
                   for b in range(B)]
            vs = [[apool.tile([H, NSUB, H], bf16, tag=f"v{b}_{i}",
                              name=f"v{b}_{i}") for i in range(NTCH)]
                  for b in range(B)]

            # ================= phase 1: projections + rope =================
            with ExitStack() as ph1:
                xpool = ph1.enter_context(tc.tile_pool(name="xpool", bufs=5))
                rtmp = ph1.enter_context(tc.tile_pool(name="rtmp", bufs=2))
                pj = ph1.enter_context(
                    tc.tile_pool(name="pj", bufs=2, space="PSUM"))
                pt = ph1.enter_context(
                    tc.tile_pool(name="pt", bufs=2, space="PSUM"))

                # very first x tile goes out ahead of all weight DMAs
                x0_t = xpool.tile([H, 8, TCH], bf16, tag="x", name="x_t")
                nc.sync.dma_start(out=x0_t, in_=xTv[:, 0:8, 0:TCH])
                nc.sync.dma_start(out=wq_t[0][0], in_=wqs[:, 0, 0:8, :])
                nc.sync.dma_start(out=wq_t[1][0], in_=wqs[:, 1, 0:8, :])
                nc.sync.dma_start(out=wk_t[0], in_=wks[:, 0:8, :])

                def rope_from_psum(psum, dst_ap, cs, sn):
                    # swap staging: halves exchanged so the mul against the
                    # (half-duplicated) rope table is one full-width op.
                    swap = rtmp.tile([H, TCH], f32, tag="swap", bufs=3,
                                     name="swap")
                    nc.vector.tensor_copy(swap[0:HH, :], psum[HH:H, :])
                    nc.vector.tensor_copy(swap[HH:H, :], psum[0:HH, :])
                    m1 = rtmp.tile([H, TCH], f32, tag="m1", name="m1")
                    m2 = rtmp.tile([H, TCH], f32, tag="m2", name="m2")
                    nc.vector.tensor_mul(m1, psum, cs)
                    nc.vector.tensor_mul(m2, swap, sn)
                    nc.vector.tensor_add(dst_ap, m1, m2)

                # staged weight-DMA drain counts, interleaved between the
                # x loads of batch 0 (emission precedes every consumer —
                # Tile deps follow emission order; kg g's weights are
                # drained right before the x group that consumes them).
                drains = {0: [[0, 3, 3, 5], [0, 4, 0, 0], [0, 4, 0, 0],
                              [0, 4, 0, 0]],
                          1: [[0, 2, 0, 0], [0, 2, 0, 0], [0, 2, 0, 0],
                              [0, 0, 0, 0]]}
                for b in range(B):
                    tb = b * T
                    for pas in range(2):
                        for tch in range(NTCH):
                            t0 = tch * TCH
                            g_ps = [pj.tile([H, TCH], f32, tag=f"g{i}",
                                            name=f"g{i}") for i in range(3)]
                            for kq in range(KPC // 8):
                                if b == 0:
                                    drain_pend(drains[pas][tch][kq])
                                if b == 0 and pas == 0 and tch == 0 \
                                        and kq == 0:
                                    x_t = x0_t
                                else:
                                    x_t = xpool.tile([H, 8, TCH], bf16,
                                                     tag="x", name="x_t")
                                    nc.sync.dma_start(
                                        out=x_t,
                                        in_=xTv[:, kq * 8:(kq + 1) * 8,
                                                tb + t0:tb + t0 + TCH])
                                last_chunk = (b == B - 1 and pas == 1
                                              and tch == NTCH - 1)
                                for kc in range(8):
                                    k = kq * 8 + kc
                                    if pas == 0:
                                        lhs = [wq_t[0][kq][:, kc, :],
                                               wq_t[1][kq][:, kc, :],
                                               wk_t[kq][:, kc, :]]
                                    elif last_chunk:
                                        # v in group 0 so its bank is the
                                        # first to free at the P1->P2
                                        # boundary (phase 2's first scores
                                        # wait on a reused PSUM bank)
                                        lhs = [wv_t[kq][:, kc, :],
                                               wq_t[2][kq][:, kc, :],
                                               wq_t[3][kq][:, kc, :]]
                                    else:
                                        lhs = [wq_t[2][kq][:, kc, :],
                                               wq_t[3][kq][:, kc, :],
                                               wv_t[kq][:, kc, :]]
                                    for gi in range(3):
                                        nc.tensor.matmul(
                                            g_ps[gi], lhs[gi], x_t[:, kc, :],
                                            start=(k == 0),
                                            stop=(k == KPC - 1),
                                            skip_group_check=True)
                            cs = cos_t[b][:, t0:t0 + TCH]
                            sn = sin_t[b][:, t0:t0 + TCH]
                            if pas == 0:
                                rope_from_psum(g_ps[2], kTs[b][tch], cs, sn)
                                rope_from_psum(g_ps[0], qTs[b][tch][:, 0, :],
                                               cs, sn)
                                rope_from_psum(g_ps[1], qTs[b][tch][:, 1, :],
                                               cs, sn)
                            else:
                                if last_chunk:
                                    g_v, g_q2, g_q3 = (g_ps[0], g_ps[1],
                                                       g_ps[2])
                                else:
                                    g_q2, g_q3, g_v = (g_ps[0], g_ps[1],
                                                       g_ps[2])
                                # v's single-copy eviction first: its PSUM
                                # bank frees soonest, which matters at the
                                # P1->P2 boundary
                                vstage = rtmp.tile([H, TCH], bf16,
                                                   tag="vstage", name="vstage")
                                nc.vector.tensor_copy(vstage, g_v)
                                for j in range(NSUB):
                                    tp = pt.tile([H, H], bf16, tag="vtp",
                                                 name="vtp")
                                    nc.tensor.transpose(
                                        tp, vstage[:, j * H:(j + 1) * H],
                                        identity)
                                    nc.vector.tensor_copy(
                                        vs[b][tch][:, j, :], tp)
                                rope_from_psum(g_q2, qTs[b][tch][:, 2, :],
                                               cs, sn)
                                rope_from_psum(g_q3, qTs[b][tch][:, 3, :],
                                               cs, sn)

            # ============= phase 2: attention + o-projection =============
            with ExitStack() as ph2:
                p2pool = ph2.enter_context(tc.tile_pool(name="p2pool", bufs=4))
                lpool = ph2.enter_context(tc.tile_pool(name="lpool", bufs=2))
                rpool = ph2.enter_context(tc.tile_pool(name="rpool", bufs=2))
                otpool = ph2.enter_context(tc.tile_pool(name="otpool", bufs=1))
                opool = ph2.enter_context(tc.tile_pool(name="opool", bufs=2))
                ps_s = ph2.enter_context(
                    tc.tile_pool(name="ps_s", bufs=3, space="PSUM"))
                ps_av = ph2.enter_context(
                    tc.tile_pool(name="ps_av", bufs=2, space="PSUM"))
                ps_lbc = ph2.enter_context(
                    tc.tile_pool(name="ps_lbc", bufs=1, space="PSUM"))
                ps_o = ph2.enter_context(
                    tc.tile_pool(name="ps_o", bufs=2, space="PSUM"))

                def attn_stream(b, outTs, qc, h):
                    """Emit one (q-chunk, head) stream: scores/AV matmuls
                    plus finalize (denominator broadcast + reciprocal +
                    normalize).  The final lsum add is a narrow diagonal
                    tile, so the broadcast matmul never waits on DVE."""
                    n_st = (qc + 1) * NSUB
                    rhs_q = qTs[b][qc][:, h, :]
                    av_ps = ps_av.tile([H, TCH], f32, tag="av",
                                       name="av_ps")
                    lsum = lpool.tile([H, TCH], bf16, tag="lsum",
                                      name="lsum")

                    def scores_block(st):
                        # diagonal-band tiles are narrowed to the causally
                        # reachable columns t >= j*128; only the 128-wide
                        # block straddling the diagonal needs masking
                        j = st - qc * NSUB
                        nw = j * H if j > 0 else 0
                        sps = ps_s.tile([H, TCH], f32, tag="s", name="sps")
                        kt = kTs[b][st // NSUB][
                            :, (st % NSUB) * H:(st % NSUB + 1) * H]
                        nc.tensor.matmul(sps[:, nw:], kt, rhs_q[:, nw:],
                                         start=True, stop=True)
                        pT2 = p2pool.tile([H, TCH], bf16, tag="p2",
                                          name="pT2")
                        nc.scalar.activation(pT2[:, nw:], sps[:, nw:], Exp,
                                             scale=C_SM)
                        if j >= 0:
                            nc.vector.tensor_mul(pT2[:, nw:nw + H],
                                                 pT2[:, nw:nw + H], wedge)
                        # softmax denominator accumulates on DVE, off the
                        # PE's critical path
                        if st == 0:
                            nc.vector.tensor_copy(lsum, pT2)
                        else:
                            nc.vector.tensor_add(lsum[:, nw:], lsum[:, nw:],
                                                 pT2[:, nw:])
                        return pT2, nw

                    def av_block(st, pT2, nw):
                        nc.tensor.matmul(
                            av_ps[:, nw:],
                            vs[b][st // NSUB][:, st % NSUB, :], pT2[:, nw:],
                            start=(st == 0), stop=(st == n_st - 1),
                            skip_group_check=True)

                    # lookahead-2: two score blocks in flight ahead of each
                    # AV so the exp/mask latency never stalls the PE
                    pending = [scores_block(0), scores_block(1)]
                    for st in range(2, n_st):
                        pending.append(scores_block(st))
                        av_block(st - 2, *pending.pop(0))
                    av_block(n_st - 2, *pending.pop(0))
                    av_block(n_st - 1, *pending.pop(0))
                    # partition-sum + broadcast of the denominator in one
                    # all-ones matmul, then fast reciprocal + normalize
                    lbc = ps_lbc.tile([H, TCH], f32, tag="lbc", name="lbc")
                    nc.tensor.matmul(lbc, ones128, lsum,
                                     start=True, stop=True)
                    rl = rpool.tile([H, TCH], f32, tag="rl", name="rl")
                    nc.vector.reciprocal_approx_fast(out=rl, in_=lbc)
                    nc.vector.tensor_mul(outTs[qc][:, h, :], av_ps, rl)

                def oproj_units(b, outTs, qc):
                    """o-projection of one q-chunk as 8 independent thunks
                    (one per (u, dh)), drained between attention streams to
                    keep the PE queue deep."""
                    tb = b * T
                    outT = outTs[qc]
                    units = []
                    for u in range(NSUB):
                        for dh in range(2):
                            def unit(u=u, dh=dh):
                                trow = tb + qc * TCH + u * H
                                # 4 PSUM evictions batched into one 4KB-row
                                # store: keeps the Sync queue off the
                                # critical path
                                o_sb = opool.tile([H, 4, TCH], bf16,
                                                  tag="osb", name="o_sb")
                                for j in range(4):
                                    dc = dh * 4 + j
                                    ops = ps_o.tile([H, TCH], f32, tag="o",
                                                    name="ops")
                                    for h in range(NHC):
                                        nc.tensor.matmul(
                                            ops,
                                            outT[:, h, u * H:(u + 1) * H],
                                            wo_t[dc // 2][:, h,
                                                          (dc % 2) * TCH:
                                                          (dc % 2 + 1) * TCH],
                                            start=(h == 0),
                                            stop=(h == NHC - 1),
                                            skip_group_check=True)
                                    nc.scalar.activation(
                                        o_sb[:, j, :], ops, Copy)
                                nc.sync.dma_start(
                                    out=o_part[trow:trow + H,
                                               dh * 2048:(dh + 1) * 2048],
                                    in_=o_sb)
                            units.append(unit)
                    return units

                opq = []        # pending o-proj units, carried across batches
                for b in range(B):
                    outTs = {qc: otpool.tile([H, NHC, TCH], bf16,
                                             tag=f"outT{qc}",
                                             name=f"outT{qc}")
                             for qc in range(NTCH)}
                    # qc3's four streams first (long streams hide each
                    # other's finalize), then qc0's short streams padded by
                    # qc3's o-projection, then the {2,1} pair alternated so
                    # qc1's short streams hide behind qc2's long ones.
                    streams = [(3, h) for h in range(NHC)]
                    streams += [(0, h) for h in range(NHC)]
                    for h in range(NHC):
                        streams.append((2, h))
                        streams.append((1, h))
                    for i, (qc, h) in enumerate(streams):
                        attn_stream(b, outTs, qc, h)
                        if i == 3:
                            opq += oproj_units(b, outTs, 3)
                        elif i == 7:
                            opq += oproj_units(b, outTs, 0)
                        for _ in range(2):
                            if opq:
                                opq.pop(0)()
                    opq += oproj_units(b, outTs, 2)
                    opq += oproj_units(b, outTs, 1)
                for unit in opq:
                    unit()

    nc.compile()
    return nc


_NC_CACHE = None


def _prep_inputs(x, wq, wk, wv, wo, positions):
    import ml_dtypes
    bf = ml_dtypes.bfloat16

    x = np.asarray(x, dtype=np.float32)
    wq = np.asarray(wq, dtype=np.float32)
    wk = np.asarray(wk, dtype=np.float32)
    wv = np.asarray(wv, dtype=np.float32)
    wo = np.asarray(wo, dtype=np.float32)
    positions = np.asarray(positions)

    xT = np.ascontiguousarray(x.reshape(TOK, D).T.astype(bf))
    # rope tables [H, TOK], duplicated across halves, sin top half negated
    fraction = 2.0 * np.arange(HH, dtype=np.float32) / H
    timescale = (THETA ** fraction).astype(np.float32)
    pos = positions.reshape(TOK).astype(np.float32)
    sinu = pos[None, :] / timescale[:, None]
    cos = np.cos(sinu).astype(np.float32)
    sin = np.sin(sinu).astype(np.float32)
    cosT = np.ascontiguousarray(np.concatenate([cos, cos], 0).astype(bf))
    sinT = np.ascontiguousarray(np.concatenate([-sin, sin], 0).astype(bf))

    in_maps = []
    for c in range(NCORES):
        wq_c = wq[c * NHC:(c + 1) * NHC]            # [4, D, H]
        wqs = np.ascontiguousarray(
            wq_c.reshape(NHC, KPC, H, H).transpose(2, 0, 1, 3).astype(bf))
        wks = np.ascontiguousarray(
            wk[c].reshape(KPC, H, H).transpose(1, 0, 2).astype(bf))
        wvs = np.ascontiguousarray(
            wv[c].reshape(KPC, H, H).transpose(1, 0, 2).astype(bf))
        wos = np.ascontiguousarray(
            wo[c * NHC:(c + 1) * NHC].transpose(1, 0, 2).astype(bf))
        in_maps.append({
            "xT": xT,
            "wqs": wqs,
            "wks": wks,
            "wvs": wvs,
            "wos": wos,
            "cosT": cosT,
            "sinT": sinT,
        })
    return in_maps


def kernel(x, wq, wk, wv, wo, positions):
    global _NC_CACHE
    from concourse.bass_utils import run_bass_kernel_spmd

    in_maps = _prep_inputs(x, wq, wk, wv, wo, positions)

    if _NC_CACHE is None:
        _NC_CACHE = _build_bass()
    nc = _NC_CACHE

    trace = os.environ.get("BASS_KERNEL_TRACE", "0") == "1"
    res = run_bass_kernel_spmd(nc, in_maps, list(range(NCORES)), trace=trace)
    global LAST_RESULTS
    LAST_RESULTS = res
    out = np.zeros((TOK, D), dtype=np.float32)
    for c in range(NCORES):
        out += np.asarray(res.results[c]["o_part"]).astype(np.float32)
    return out.reshape(B, T, D)


LAST_RESULTS = None


# revision 32
# speedup vs baseline: 1.0060x; 1.0024x over previous
"""GQA causal-attention prefill kernel for Trainium2, tensor-parallel over 8 NeuronCores.

Reference semantics: q/k/v projections + RoPE + causal GQA attention +
output projection, B=2, T=2048, D=4096, 32 q heads, 8 kv heads, head_dim
128.  Core c owns q heads [4c, 4c+4), kv head c and the matching wo
slice; each core computes a full-shape partial output o_part and the
host sums the 8 partials (the tensor-parallel all-reduce).

Everything on the PE runs in bf16 (fp32 PSUM accumulation); measured
end-to-end max-rel error vs the fp32 reference is ~4e-3, well inside the
2e-2 gate, and bf16 halves DMA traffic, halves SBUF footprint (so all
weights + both batches' activations stay resident) and unlocks the
2-4x DVE 16-bit modes for the softmax bookkeeping.

Structure (emission order = engine program order):
  P1(b0), P1(b1):  projections + rope, TWO passes per batch over x
      (pass A: q0,q1,k; pass B: q2,q3,v).  3 accumulation groups x
      bufs=2 PSUM banks -> evictions of chunk c overlap the full 20us
      K-sweep of chunk c+1, so the PE never waits on a bank.  x is read
      twice (bf16 makes the 2x stream fit in HBM bandwidth); weights
      are loaded once up front, in k-group tiles so the first matmul
      only waits for ~1.5MB.
  P2(b0), P2(b1):  attention + o-projection per 512-token q-chunk.
      Scores transposed (sT = kT.T @ qT) so AV contracts s on the
      partition dim.  Softmax denominator comes from DVE adds of the
      exp tiles (off the PE) + ONE all-ones [128,128] matmul per
      (chunk, head) that sums over partitions AND broadcasts in one
      shot; 1/l via the fast custom-DVE reciprocal.  q-chunks are
      processed in pairs {3,0},{2,1} with heads interleaved so each
      stream's finalize chain hides behind a long stream's matmuls,
      and the o-projection of finished chunks is emitted between
      streams to keep the PE queue deep.
"""

import os
import sys

sys.path.insert(0, "/opt/trn_rl_repo")

import numpy as np

B = 2
T = 2048
TOK = B * T
D = 4096
NQ = 32
NKV = 8
H = 128
HH = H // 2
THETA = 10000.0
NCORES = 8
NHC = NQ // NCORES          # q heads per core (4)
KPC = D // H                # contraction chunks of 128 over D (32)
KG = 4                      # k-groups per weight tensor (8 chunks each)
TCH = 512                   # token chunk
NTCH = T // TCH             # 4 token chunks per batch
NSUB = TCH // H             # 4 s-subtiles per chunk
C_SM = 1.0 / np.sqrt(H)     # softmax scale


def _build_bass():
    import concourse.bacc as bacc
    import concourse.mybir as mybir
    import concourse.tile as tile
    from concourse.masks import make_identity
    from contextlib import ExitStack

    f32 = mybir.dt.float32
    bf16 = mybir.dt.bfloat16
    Exp = mybir.ActivationFunctionType.Exp
    Copy = mybir.ActivationFunctionType.Copy

    nc = bacc.Bacc("TRN2", target_bir_lowering=False, debug=False,
                   num_devices=NCORES)

    xT = nc.declare_dram_parameter("xT", [D, TOK], bf16, isOutput=False)
    # host pre-shuffled so every DMA row is >=2KB contiguous:
    # wqs[p, h, c, m] = wq[h, c*128+p, m]
    wqs = nc.declare_dram_parameter("wqs", [H, NHC, KPC, H], bf16,
                                    isOutput=False)
    wks = nc.declare_dram_parameter("wks", [H, KPC, H], bf16, isOutput=False)
    wvs = nc.declare_dram_parameter("wvs", [H, KPC, H], bf16, isOutput=False)
    # wos[p, h, d] = wo[h, p, d]
    wos = nc.declare_dram_parameter("wos", [H, NHC, D], bf16, isOutput=False)
    # rope tables duplicated across partition halves; sinT's TOP half is
    # NEGATED on the host so rope is out = direct*cosT + swap*sinT for all
    # 128 partitions in one mul+mul+add.
    cosT = nc.declare_dram_parameter("cosT", [H, TOK], bf16, isOutput=False)
    sinT = nc.declare_dram_parameter("sinT", [H, TOK], bf16, isOutput=False)
    o_part = nc.declare_dram_parameter("o_part", [TOK, D], bf16, isOutput=True)
    # x viewed as [p, kchunk, t] so one DMA start can fetch 4 k-chunks
    # (each dma_start costs ~600ns of serial Sync-sequencer time; the
    # un-batched version saturated that queue)
    xTv = xT.rearrange("(c p) t -> p c t", p=H)

    with tile.TileContext(nc) as tc:
        with ExitStack() as top:
            consts = top.enter_context(tc.tile_pool(name="consts", bufs=1))
            identity = consts.tile([H, H], bf16)
            make_identity(nc, identity)
            ones128 = consts.tile([H, H], bf16, tag="ones128")
            nc.vector.memset(ones128, 1.0)
            # 0/1 causal wedge for the 128x128 block that straddles the
            # diagonal: wedge[s, t'] = 1 iff t' >= s.  Blocks left of it are
            # skipped entirely (matmuls narrowed), blocks right of it are
            # all-ones (no mask needed).
            wedge = consts.tile([H, H], bf16, tag="wedge")
            nc.vector.memset(wedge, 1.0)
            nc.gpsimd.affine_select(
                out=wedge, in_=wedge,
                compare_op=mybir.AluOpType.is_ge,
                fill=0.0,
                base=0,
                pattern=[[1, H]],
                channel_multiplier=-1,
            )

            # ---- weights: loaded once, staged so x streaming stays ahead ----
            wpool = top.enter_context(tc.tile_pool(name="wpool", bufs=1))
            wq_t = [[wpool.tile([H, 8, H], bf16, tag=f"wq{h}_{g}",
                                name=f"wq{h}_{g}") for g in range(KG)]
                    for h in range(NHC)]
            wk_t = [wpool.tile([H, 8, H], bf16, tag=f"wk{g}", name=f"wk{g}")
                    for g in range(KG)]
            wv_t = [wpool.tile([H, 8, H], bf16, tag=f"wv{g}", name=f"wv{g}")
                    for g in range(KG)]
            wo_t = [wpool.tile([H, NHC, 1024], bf16, tag=f"wo{dq}",
                               name=f"wo{dq}") for dq in range(4)]
            cos_t = [wpool.tile([H, T], bf16, tag=f"cos{b}", name=f"cos{b}")
                     for b in range(B)]
            sin_t = [wpool.tile([H, T], bf16, tag=f"sin{b}", name=f"sin{b}")
                     for b in range(B)]

            # immediately needed: pass-A k-group 0; everything else is
            # drained between x loads so the first x tile isn't queued
            # behind megabytes of weights.  (The first x tile itself is
            # issued before even these, inside phase 1.)
            pend = []
            for g in range(1, KG):
                pend.append((wq_t[0][g], wqs[:, 0, g * 8:(g + 1) * 8, :]))
                pend.append((wq_t[1][g], wqs[:, 1, g * 8:(g + 1) * 8, :]))
                pend.append((wk_t[g], wks[:, g * 8:(g + 1) * 8, :]))
            pend.append((cos_t[0], cosT[:, 0:T]))
            pend.append((sin_t[0], sinT[:, 0:T]))
            for g in range(KG):
                pend.append((wq_t[2][g], wqs[:, 2, g * 8:(g + 1) * 8, :]))
                pend.append((wq_t[3][g], wqs[:, 3, g * 8:(g + 1) * 8, :]))
                pend.append((wv_t[g], wvs[:, g * 8:(g + 1) * 8, :]))
            pend.append((cos_t[1], cosT[:, T:TOK]))
            pend.append((sin_t[1], sinT[:, T:TOK]))
            for dq in range(4):
                pend.append((wo_t[dq], wos[:, :, dq * 1024:(dq + 1) * 1024]))

            def drain_pend(n):
                for _ in range(n):
                    if pend:
                        dst, src = pend.pop(0)
                        nc.sync.dma_start(out=dst, in_=src)

            # ---- activations, both batches resident (bf16) ----
            apool = top.enter_context(tc.tile_pool(name="apool", bufs=1))
            qTs = [[apool.tile([H, NHC, TCH], bf16, tag=f"qT{b}_{i}",
                               name=f"qT{b}_{i}") for i in range(NTCH)]
                   for b in range(B)]
            kTs = [[apool.tile([H, TCH], bf16, tag=f"kT{b}_{i}",
                               name=f"kT{b}_{i}") for i in range(NTCH)]
                   for b in range(B)]
            vs = [[apool.tile([H, NSUB, H], bf16, tag=f"v{b}_{i}",
                              name=f"v{b}_{i}") for i in range(NTCH)]
                  for b in range(B)]

            # ================= phase 1: projections + rope =================
            with ExitStack() as ph1:
                xpool = ph1.enter_context(tc.tile_pool(name="xpool", bufs=5))
                rtmp = ph1.enter_context(tc.tile_pool(name="rtmp", bufs=2))
                pj = ph1.enter_context(
                    tc.tile_pool(name="pj", bufs=2, space="PSUM"))
                pt = ph1.enter_context(
                    tc.tile_pool(name="pt", bufs=2, space="PSUM"))

                # very first x tile goes out ahead of all weight DMAs
                x0_t = xpool.tile([H, 8, TCH], bf16, tag="x", name="x_t")
                nc.sync.dma_start(out=x0_t, in_=xTv[:, 0:8, 0:TCH])
                nc.sync.dma_start(out=wq_t[0][0], in_=wqs[:, 0, 0:8, :])
                nc.sync.dma_start(out=wq_t[1][0], in_=wqs[:, 1, 0:8, :])
                nc.sync.dma_start(out=wk_t[0], in_=wks[:, 0:8, :])

                def rope_from_psum(psum, dst_ap, cs, sn):
                    # swap staging: halves exchanged so the mul against the
                    # (half-duplicated) rope table is one full-width op.
                    swap = rtmp.tile([H, TCH], f32, tag="swap", bufs=3,
                                     name="swap")
                    nc.vector.tensor_copy(swap[0:HH, :], psum[HH:H, :])
                    nc.vector.tensor_copy(swap[HH:H, :], psum[0:HH, :])
                    m1 = rtmp.tile([H, TCH], f32, tag="m1", name="m1")
                    m2 = rtmp.tile([H, TCH], f32, tag="m2", name="m2")
                    nc.vector.tensor_mul(m1, psum, cs)
                    nc.vector.tensor_mul(m2, swap, sn)
                    nc.vector.tensor_add(dst_ap, m1, m2)

                # staged weight-DMA drain counts, interleaved between the
                # x loads of batch 0 (emission precedes every consumer —
                # Tile deps follow emission order; kg g's weights are
                # drained right before the x group that consumes them).
                drains = {0: [[0, 3, 3, 5], [0, 4, 0, 0], [0, 4, 0, 0],
                              [0, 4, 0, 0]],
                          1: [[0, 2, 0, 0], [0, 2, 0, 0], [0, 2, 0, 0],
                              [0, 0, 0, 0]]}
                for b in range(B):
                    tb = b * T
                    for pas in range(2):
                        for tch in range(NTCH):
                            t0 = tch * TCH
                            g_ps = [pj.tile([H, TCH], f32, tag=f"g{i}",
                                            name=f"g{i}") for i in range(3)]
                            for kq in range(KPC // 8):
                                if b == 0:
                                    drain_pend(drains[pas][tch][kq])
                                if b == 0 and pas == 0 and tch == 0 \
                                        and kq == 0:
                                    x_t = x0_t
                                else:
                                    x_t = xpool.tile([H, 8, TCH], bf16,
                                                     tag="x", name="x_t")
                                    nc.sync.dma_start(
                                        out=x_t,
                                        in_=xTv[:, kq * 8:(kq + 1) * 8,
                                                tb + t0:tb + t0 + TCH])
                                last_chunk = (b == B - 1 and pas == 1
                                              and tch == NTCH - 1)
                                for kc in range(8):
                                    k = kq * 8 + kc
                                    if pas == 0:
                                        lhs = [wq_t[0][kq][:, kc, :],
                                               wq_t[1][kq][:, kc, :],
                                               wk_t[kq][:, kc, :]]
                                    elif last_chunk:
                                        # v in group 0 so its bank is the
                                        # first to free at the P1->P2
                                        # boundary (phase 2's first scores
                                        # wait on a reused PSUM bank)
                                        lhs = [wv_t[kq][:, kc, :],
                                               wq_t[2][kq][:, kc, :],
                                               wq_t[3][kq][:, kc, :]]
                                    else:
                                        lhs = [wq_t[2][kq][:, kc, :],
                                               wq_t[3][kq][:, kc, :],
                                               wv_t[kq][:, kc, :]]
                                    for gi in range(3):
                                        nc.tensor.matmul(
                                            g_ps[gi], lhs[gi], x_t[:, kc, :],
                                            start=(k == 0),
                                            stop=(k == KPC - 1),
                                            skip_group_check=True)
                            cs = cos_t[b][:, t0:t0 + TCH]
                            sn = sin_t[b][:, t0:t0 + TCH]
                            if pas == 0:
                                rope_from_psum(g_ps[2], kTs[b][tch], cs, sn)
                                rope_from_psum(g_ps[0], qTs[b][tch][:, 0, :],
                                               cs, sn)
                                rope_from_psum(g_ps[1], qTs[b][tch][:, 1, :],
                                               cs, sn)
                            else:
                                if last_chunk:
                                    g_v, g_q2, g_q3 = (g_ps[0], g_ps[1],
                                                       g_ps[2])
                                else:
                                    g_q2, g_q3, g_v = (g_ps[0], g_ps[1],
                                                       g_ps[2])
                                # v's single-copy eviction first: its PSUM
                                # bank frees soonest, which matters at the
                                # P1->P2 boundary
                                vstage = rtmp.tile([H, TCH], bf16,
                                                   tag="vstage", name="vstage")
                                nc.vector.tensor_copy(vstage, g_v)
                                for j in range(NSUB):
                                    tp = pt.tile([H, H], bf16, tag="vtp",
                                                 name="vtp")
                                    nc.tensor.transpose(
                                        tp, vstage[:, j * H:(j + 1) * H],
                                        identity)
                                    nc.vector.tensor_copy(
                                        vs[b][tch][:, j, :], tp)
                                rope_from_psum(g_q2, qTs[b][tch][:, 2, :],
                                               cs, sn)
                                rope_from_psum(g_q3, qTs[b][tch][:, 3, :],
                                               cs, sn)

            # ============= phase 2: attention + o-projection =============
            with ExitStack() as ph2:
                p2pool = ph2.enter_context(tc.tile_pool(name="p2pool", bufs=5))
                lpool = ph2.enter_context(tc.tile_pool(name="lpool", bufs=2))
                rpool = ph2.enter_context(tc.tile_pool(name="rpool", bufs=2))
                otpool = ph2.enter_context(tc.tile_pool(name="otpool", bufs=1))
                opool = ph2.enter_context(tc.tile_pool(name="opool", bufs=2))
                ps_s = ph2.enter_context(
                    tc.tile_pool(name="ps_s", bufs=3, space="PSUM"))
                ps_av = ph2.enter_context(
                    tc.tile_pool(name="ps_av", bufs=2, space="PSUM"))
                ps_lbc = ph2.enter_context(
                    tc.tile_pool(name="ps_lbc", bufs=1, space="PSUM"))
                ps_o = ph2.enter_context(
                    tc.tile_pool(name="ps_o", bufs=2, space="PSUM"))

                def attn_stream(b, outTs, qc, h):
                    """Emit one (q-chunk, head) stream: scores/AV matmuls
                    plus finalize (denominator broadcast + reciprocal +
                    normalize).  The final lsum add is a narrow diagonal
                    tile, so the broadcast matmul never waits on DVE."""
                    n_st = (qc + 1) * NSUB
                    rhs_q = qTs[b][qc][:, h, :]
                    av_ps = ps_av.tile([H, TCH], f32, tag="av",
                                       name="av_ps")
                    lsum = lpool.tile([H, TCH], bf16, tag="lsum",
                                      name="lsum")

                    def scores_block(st):
                        # diagonal-band tiles are narrowed to the causally
                        # reachable columns t >= j*128; only the 128-wide
                        # block straddling the diagonal needs masking
                        j = st - qc * NSUB
                        nw = j * H if j > 0 else 0
                        sps = ps_s.tile([H, TCH], f32, tag="s", name="sps")
                        kt = kTs[b][st // NSUB][
                            :, (st % NSUB) * H:(st % NSUB + 1) * H]
                        nc.tensor.matmul(sps[:, nw:], kt, rhs_q[:, nw:],
                                         start=True, stop=True)
                        pT2 = p2pool.tile([H, TCH], bf16, tag="p2",
                                          name="pT2")
                        nc.scalar.activation(pT2[:, nw:], sps[:, nw:], Exp,
                                             scale=C_SM)
                        if j >= 0:
                            nc.vector.tensor_mul(pT2[:, nw:nw + H],
                                                 pT2[:, nw:nw + H], wedge)
                        # softmax denominator accumulates on DVE, off the
                        # PE's critical path
                        if st == 0:
                            nc.vector.tensor_copy(lsum, pT2)
                        else:
                            nc.vector.tensor_add(lsum[:, nw:], lsum[:, nw:],
                                                 pT2[:, nw:])
                        return pT2, nw

                    def av_block(st, pT2, nw):
                        nc.tensor.matmul(
                            av_ps[:, nw:],
                            vs[b][st // NSUB][:, st % NSUB, :], pT2[:, nw:],
                            start=(st == 0), stop=(st == n_st - 1),
                            skip_group_check=True)

                    # lookahead-3: three score blocks in flight ahead of
                    # each AV — the exp chain (sem + ~430ns ACT + sem) is
                    # longer than two matmuls' worth of cover, so AV was
                    # still eating ~200ns waits at lookahead-2.  The three
                    # score PSUM banks rotate fine: a bank frees after its
                    # exp, well before the 3rd-next score needs it.
                    pending = [scores_block(0), scores_block(1),
                               scores_block(2)]
                    for st in range(3, n_st):
                        pending.append(scores_block(st))
                        av_block(st - 3, *pending.pop(0))
                    for back in range(3, 0, -1):
                        av_block(n_st - back, *pending.pop(0))
                    # partition-sum + broadcast of the denominator in one
                    # all-ones matmul, then fast reciprocal + normalize
                    lbc = ps_lbc.tile([H, TCH], f32, tag="lbc", name="lbc")
                    nc.tensor.matmul(lbc, ones128, lsum,
                                     start=True, stop=True)
                    rl = rpool.tile([H, TCH], f32, tag="rl", name="rl")
                    nc.vector.reciprocal_approx_fast(out=rl, in_=lbc)
                    nc.vector.tensor_mul(outTs[qc][:, h, :], av_ps, rl)

                def oproj_units(b, outTs, qc):
                    """o-projection of one q-chunk as 8 independent thunks
                    (one per (u, dh)), drained between attention streams to
                    keep the PE queue deep."""
                    tb = b * T
                    outT = outTs[qc]
                    units = []
                    for u in range(NSUB):
                        for dh in range(2):
                            def unit(u=u, dh=dh):
                                trow = tb + qc * TCH + u * H
                                # 4 PSUM evictions batched into one 4KB-row
                                # store: keeps the Sync queue off the
                                # critical path
                                o_sb = opool.tile([H, 4, TCH], bf16,
                                                  tag="osb", name="o_sb")
                                for j in range(4):
                                    dc = dh * 4 + j
                                    ops = ps_o.tile([H, TCH], f32, tag="o",
                                                    name="ops")
                                    for h in range(NHC):
                                        nc.tensor.matmul(
                                            ops,
                                            outT[:, h, u * H:(u + 1) * H],
                                            wo_t[dc // 2][:, h,
                                                          (dc % 2) * TCH:
                                                          (dc % 2 + 1) * TCH],
                                            start=(h == 0),
                                            stop=(h == NHC - 1),
                                            skip_group_check=True)
                                    nc.scalar.activation(
                                        o_sb[:, j, :], ops, Copy)
                                nc.sync.dma_start(
                                    out=o_part[trow:trow + H,
                                               dh * 2048:(dh + 1) * 2048],
                                    in_=o_sb)
                            units.append(unit)
                    return units

                opq = []        # pending o-proj units, carried across batches
                for b in range(B):
                    outTs = {qc: otpool.tile([H, NHC, TCH], bf16,
                                             tag=f"outT{qc}",
                                             name=f"outT{qc}")
                             for qc in range(NTCH)}
                    # qc3's four streams first (long streams hide each
                    # other's finalize), then qc0's short streams padded by
                    # qc3's o-projection, then the {2,1} pair alternated so
                    # qc1's short streams hide behind qc2's long ones.
                    streams = [(3, h) for h in range(NHC)]
                    streams += [(0, h) for h in range(NHC)]
                    for h in range(NHC):
                        streams.append((2, h))
                        streams.append((1, h))
                    for i, (qc, h) in enumerate(streams):
                        attn_stream(b, outTs, qc, h)
                        if i == 3:
                            opq += oproj_units(b, outTs, 3)
                        elif i == 7:
                            opq += oproj_units(b, outTs, 0)
                        for _ in range(2):
                            if opq:
                                opq.pop(0)()
                    opq += oproj_units(b, outTs, 2)
                    opq += oproj_units(b, outTs, 1)
                for unit in opq:
                    unit()

    nc.compile()
    return nc


_NC_CACHE = None


def _prep_inputs(x, wq, wk, wv, wo, positions):
    import ml_dtypes
    bf = ml_dtypes.bfloat16

    x = np.asarray(x, dtype=np.float32)
    wq = np.asarray(wq, dtype=np.float32)
    wk = np.asarray(wk, dtype=np.float32)
    wv = np.asarray(wv, dtype=np.float32)
    wo = np.asarray(wo, dtype=np.float32)
    positions = np.asarray(positions)

    xT = np.ascontiguousarray(x.reshape(TOK, D).T.astype(bf))
    # rope tables [H, TOK], duplicated across halves, sin top half negated
    fraction = 2.0 * np.arange(HH, dtype=np.float32) / H
    timescale = (THETA ** fraction).astype(np.float32)
    pos = positions.reshape(TOK).astype(np.float32)
    sinu = pos[None, :] / timescale[:, None]
    cos = np.cos(sinu).astype(np.float32)
    sin = np.sin(sinu).astype(np.float32)
    cosT = np.ascontiguousarray(np.concatenate([cos, cos], 0).astype(bf))
    sinT = np.ascontiguousarray(np.concatenate([-sin, sin], 0).astype(bf))

    in_maps = []
    for c in range(NCORES):
        wq_c = wq[c * NHC:(c + 1) * NHC]            # [4, D, H]
        wqs = np.ascontiguousarray(
            wq_c.reshape(NHC, KPC, H, H).transpose(2, 0, 1, 3).astype(bf))
        wks = np.ascontiguousarray(
            wk[c].reshape(KPC, H, H).transpose(1, 0, 2).astype(bf))
        wvs = np.ascontiguousarray(
            wv[c].reshape(KPC, H, H).transpose(1, 0, 2).astype(bf))
        wos = np.ascontiguousarray(
            wo[c * NHC:(c + 1) * NHC].transpose(1, 0, 2).astype(bf))
        in_maps.append({
            "xT": xT,
            "wqs": wqs,
            "wks": wks,
            "wvs": wvs,
            "wos": wos,
            "cosT": cosT,
            "sinT": sinT,
        })
    return in_maps


def kernel(x, wq, wk, wv, wo, positions):
    global _NC_CACHE
    from concourse.bass_utils import run_bass_kernel_spmd

    in_maps = _prep_inputs(x, wq, wk, wv, wo, positions)

    if _NC_CACHE is None:
        _NC_CACHE = _build_bass()
    nc = _NC_CACHE

    trace = os.environ.get("BASS_KERNEL_TRACE", "0") == "1"
    res = run_bass_kernel_spmd(nc, in_maps, list(range(NCORES)), trace=trace)
    global LAST_RESULTS
    LAST_RESULTS = res
    out = np.zeros((TOK, D), dtype=np.float32)
    for c in range(NCORES):
        out += np.asarray(res.results[c]["o_part"]).astype(np.float32)
    return out.reshape(B, T, D)


LAST_RESULTS = None
